# revision 39
# baseline (speedup 1.0000x reference)
"""Grouped MoE MLP (64 experts) on 8 Trainium2 NeuronCores.

Strategy: expert parallelism. Each core owns 8 experts (size-sorted "snake"
assignment so every core gets the same per-slot padded token capacity and the
padding is tight). Host gathers each core's tokens into per-expert padded
blocks laid out transposed so both matmuls stream tokens as the moving
operand:

    hT[f, t]   = w1t[e] (stationary, [h,f] tiles) @ xT (moving, [h, t])
    hT         = gelu(hT / S1)                (ScalarE, PSUM f32 -> SBUF bf16)
    outT[o, t] = w2[e] (stationary, [f,o] tiles) @ hT (moving, [f, t])

w1 is pre-scaled by S1 and stored fp8e3 (e3m4) — the matmul runs with an
fp8e3 stationary operand against a bf16 moving operand at full bf16 rate,
halving w1's HBM traffic; the 1/S1 unscale folds into the gelu activation's
scale. w2 stays bf16. All DRAM layouts are flat per-partition-contiguous and
exactly match the SBUF tiles, so every transfer is one large contiguous DMA.
Output is written bf16 and upcast on host.
"""

import numpy as np

NCORES = 8
SLOTS = 8  # experts per core
NE = 64
H = 1024
F = 2048
T = 16384
P = 128
KO = H // P  # 8  k-tiles for mm1 (contraction over H)
FO = F // P  # 16 f-tiles (mm1 output tiles / mm2 contraction)
OO = H // P  # 8  output h-tiles for mm2
NMAX = 512  # max moving-operand length (one fp32 PSUM bank)

ACT_FN = "Gelu"  # overridable for CoreSim tests (Gelu not implemented there)
W1_FP8 = True  # w1 stationary operand in fp8e3 (e3m4), else bf16

_prog_cache = {}


def _build_program(C, s1_inv):
    """Build the SPMD Bass program for per-slot token capacities C."""
    from contextlib import ExitStack

    import concourse.tile as tile
    from concourse import bacc, mybir
    from concourse.bass import MemorySpace

    bf16 = mybir.dt.bfloat16
    f8e3 = mybir.dt.float8e3
    f32 = mybir.dt.float32
    u8 = mybir.dt.uint8
    w1_dt = f8e3 if W1_FP8 else bf16
    w1_esz = 1 if W1_FP8 else 2
    Cmax = int(max(C))
    NT = min(NMAX, Cmax)  # h/o tile column capacity
    XLEN = KO * int(sum(C))
    OLEN = OO * int(sum(C))
    L = KO * F
    # boot bundle: slot 0's first w1 chunk + its token block in one DMA
    BL = (L // 8) * w1_esz
    BOOTLEN = BL + KO * int(C[0]) * 2

    nc = bacc.Bacc("TRN2", target_bir_lowering=False, debug=False, num_devices=NCORES)
    w1_d = nc.dram_tensor("w1q", [P, SLOTS * KO * F], w1_dt, kind="ExternalInput").ap()
    w2_d = nc.dram_tensor("w2", [P, SLOTS * FO * H], bf16, kind="ExternalInput").ap()
    x_d = nc.dram_tensor("xT", [P, XLEN], bf16, kind="ExternalInput").ap()
    bt_d = nc.dram_tensor("boot", [P, BOOTLEN], u8, kind="ExternalInput").ap()
    o_d = nc.dram_tensor("outT", [P, OLEN], bf16, kind="ExternalOutput").ap()

    act_fn = getattr(mybir.ActivationFunctionType, ACT_FN)

    with tile.TileContext(nc) as tc, ExitStack() as ctx:
        w1_pool = ctx.enter_context(tc.tile_pool(name="w1", bufs=2))
        w2_pool = ctx.enter_context(tc.tile_pool(name="w2", bufs=2))
        x_pool = ctx.enter_context(tc.tile_pool(name="x", bufs=2))
        h_pool = ctx.enter_context(tc.tile_pool(name="h", bufs=2))
        o_pool = ctx.enter_context(tc.tile_pool(name="o", bufs=2))
        ph_pool = ctx.enter_context(
            tc.tile_pool(name="ph", bufs=3, space=MemorySpace.PSUM)
        )
        po_pool = ctx.enter_context(
            tc.tile_pool(name="po", bufs=3, space=MemorySpace.PSUM)
        )

        # PE warmup: dependency-free matmuls on scratch tiles so the HAM
        # clock-gate releases (1.2 -> 2.4 GHz) before the first real data
        # arrives; results are never read
        warm_pool = ctx.enter_context(tc.tile_pool(name="wm", bufs=1))
        bt_pool = ctx.enter_context(tc.tile_pool(name="bt", bufs=1))
        wm_sb = warm_pool.tile([P, 2 * P], bf16, tag="wm")
        nc.vector.memset(wm_sb, 0.0)
        pw = ph_pool.tile([P, NMAX], f32, tag="ph")
        for _ in range(25):
            nc.tensor.matmul(
                pw[:, :256], wm_sb[:, :P], wm_sb[:, : 2 * P], start=True, stop=True
            )

        xoff = 0
        ooff = 0
        for j in range(SLOTS):
            Cj = int(C[j])
            # w1 is laid out fo-major ([fo, k, fi] per partition) and loaded in
            # chunks so mm1 can start as soon as the first one lands; slot 0
            # gets its first w1 chunk + token block fused into one boot DMA
            # (one issue + one completion on the cold-start critical path)
            w1_sb = w1_pool.tile([P, KO * F], w1_dt, tag="w1")
            if j == 0:
                boot_sb = bt_pool.tile([P, BOOTLEN], u8, tag="bt")
                nc.sync.dma_start(boot_sb, bt_d)
                x_sb = None
                bounds = [L // 8, L // 4, L // 2, L]
            else:
                x_sb = x_pool.tile([P, KO * Cmax], bf16, tag="x")
                bounds = [0, L // 4, L // 2, L]
                nc.sync.dma_start(
                    w1_sb[:, : bounds[1]], w1_d[:, j * L : j * L + bounds[1]]
                )
                nc.sync.dma_start(
                    x_sb[:, : KO * Cj], x_d[:, xoff : xoff + KO * Cj]
                )
                bounds = bounds[1:]
            for cc, ce in zip(bounds, bounds[1:]):
                nc.sync.dma_start(
                    w1_sb[:, cc:ce], w1_d[:, j * L + cc : j * L + ce]
                )

            def w1_ap(fo, k, _j=j, _w1=w1_sb, _bt=boot_sb if j == 0 else None):
                idx = (fo * KO + k) * P
                if _j == 0 and idx < L // 8:
                    return _bt[:, idx * w1_esz : (idx + P) * w1_esz].bitcast(w1_dt)
                return _w1[:, idx : idx + P]

            def x_ap(col, n, _j=j, _Cj=Cj, _x=x_sb, _bt=boot_sb if j == 0 else None):
                if _j == 0:
                    return _bt[:, BL + col * 2 : BL + (col + n) * 2].bitcast(bf16)
                return _x[:, col : col + n]

            # w2 stays on the sync ring: FIFO order doubles as priority —
            # a second ring would round-robin packets and starve the next
            # slot's urgent x/w1 prefetch (measured +10us)
            w2_sb = w2_pool.tile([P, FO * H], bf16, tag="w2")
            nc.sync.dma_start(w2_sb, w2_d[:, j * FO * H : (j + 1) * FO * H])

            for nb in range(0, Cj, NMAX):
                NB = min(NMAX, Cj - nb)
                h_sb = h_pool.tile([P, FO * NT], bf16, tag="h")
                for fo in range(FO):
                    ph = ph_pool.tile([P, NMAX], f32, tag="ph")
                    for k in range(KO):
                        nc.tensor.matmul(
                            ph[:, :NB],
                            w1_ap(fo, k),
                            x_ap(k * Cj + nb, NB),
                            start=(k == 0),
                            stop=(k == KO - 1),
                        )
                    nc.scalar.activation(
                        h_sb[:, fo * NT : fo * NT + NB],
                        ph[:, :NB],
                        act_fn,
                        scale=float(s1_inv),
                    )
                o_sb = o_pool.tile([P, OO * NT], bf16, tag="o")
                # on the final chunk, flush the first half of the outputs
                # early so the last DMA overlaps the tail of mm2
                last = j == SLOTS - 1 and nb + NMAX >= Cj
                for oo in range(OO):
                    po = po_pool.tile([P, NMAX], f32, tag="po")
                    for fo in range(FO):
                        nc.tensor.matmul(
                            po[:, :NB],
                            w2_sb[:, fo * H + oo * P : fo * H + (oo + 1) * P],
                            h_sb[:, fo * NT : fo * NT + NB],
                            start=(fo == 0),
                            stop=(fo == FO - 1),
                        )
                    nc.vector.tensor_copy(o_sb[:, oo * NB : (oo + 1) * NB], po[:, :NB])
                    # flush outputs incrementally on the final chunk so the
                    # last DMA is tiny and overlaps the tail of mm2
                    if last and oo >= OO // 2 - 1 and oo < OO - 1:
                        nc.sync.dma_start(
                            o_d[
                                :,
                                ooff + nb * OO + oo * NB : ooff + nb * OO + (oo + 1) * NB,
                            ]
                            if oo > OO // 2 - 1
                            else o_d[:, ooff + nb * OO : ooff + nb * OO + OO // 2 * NB],
                            o_sb[:, oo * NB : (oo + 1) * NB]
                            if oo > OO // 2 - 1
                            else o_sb[:, : OO // 2 * NB],
                        )
                if last:
                    nc.sync.dma_start(
                        o_d[:, ooff + nb * OO + (OO - 1) * NB : ooff + nb * OO + OO * NB],
                        o_sb[:, (OO - 1) * NB : OO * NB],
                    )
                else:
                    nc.sync.dma_start(
                        o_d[:, ooff + nb * OO : ooff + nb * OO + OO * NB],
                        o_sb[:, : OO * NB],
                    )
            xoff += KO * Cj
            ooff += OO * Cj

    nc.compile()
    return nc


def _get_program(C, s1_inv):
    key = (tuple(int(c) for c in C), float(s1_inv))
    if key not in _prog_cache:
        _prog_cache[key] = _build_program(key[0], key[1])
    return _prog_cache[key]


def plan(sizes):
    """Expert->core/slot assignment + slot capacities from token counts."""
    sizes = np.asarray(sizes, np.int64)
    assert sizes.shape == (NE,) and sizes.sum() == T
    order = np.argsort(-sizes, kind="stable")  # descending
    expert_of = [
        [int(order[s * NCORES + c]) for s in range(SLOTS)] for c in range(NCORES)
    ]
    C = []
    for s in range(SLOTS):
        m = max(int(sizes[order[s * NCORES + c]]) for c in range(NCORES))
        C.append(max(16, int(m)))  # min 16
    offs = np.concatenate([[0], np.cumsum(C)]).astype(np.int64)
    return expert_of, C, offs


def _w1_scale(w1):
    """Largest power-of-2 scale keeping |w1*S1| safely inside e3m4 range."""
    m = float(np.abs(w1).max())
    if m == 0.0:
        return 1.0
    return float(2.0 ** np.floor(np.log2(15.0 / m)))


def prepare_inputs(x, w1, w2, sizes, expert_of, C, offs, s1):
    """Host-side shard/pad/transpose/cast. Returns per-core input maps."""
    import ml_dtypes

    bf16 = ml_dtypes.bfloat16
    f8e3 = ml_dtypes.float8_e3m4
    x = np.asarray(x, np.float32)
    w1 = np.asarray(w1, np.float32)
    w2 = np.asarray(w2, np.float32)
    tok_offs = np.concatenate([[0], np.cumsum(sizes)]).astype(np.int64)

    in_maps = []
    for c in range(NCORES):
        experts = expert_of[c]
        # w1: [S,F,H] -> [S,H,F] -> [S,KO,P_hi,FO,P_fi] -> fo-major
        # per-partition layout [S, FO, KO, P_fi] to match the kernel's slices
        w1c = w1[experts].transpose(0, 2, 1).reshape(SLOTS, KO, P, FO, P)
        w1c = np.ascontiguousarray(w1c.transpose(2, 0, 3, 1, 4)).reshape(P, -1)
        if W1_FP8:
            w1q = (w1c * s1).astype(f8e3)
        else:
            w1q = (w1c * s1).astype(bf16)
        # w2: [S,F,H] -> [S,FO,P,H] -> [P, S*FO*H]
        w2c = w2[experts].reshape(SLOTS, FO, P, H)
        w2c = (
            np.ascontiguousarray(w2c.transpose(2, 0, 1, 3)).reshape(P, -1).astype(bf16)
        )
        # x: per-slot [Cj,H] padded -> [H,Cj] -> [KO,P,Cj] -> [P, KO*Cj], concat
        xparts = []
        for s, e in enumerate(experts):
            n = int(sizes[e])
            Cj = int(C[s])
            xe = np.zeros((Cj, H), np.float32)
            xe[:n] = x[tok_offs[e] : tok_offs[e] + n]
            xparts.append(xe.T.reshape(KO, P, Cj).transpose(1, 0, 2).reshape(P, -1))
        xc = np.ascontiguousarray(np.concatenate(xparts, axis=1)).astype(bf16)
        # boot bundle: w1 slot-0 first chunk bytes || slot-0 token block bytes
        L = KO * F
        boot = np.ascontiguousarray(
            np.concatenate(
                [
                    w1q[:, : L // 8].view(np.uint8),
                    xc[:, : KO * int(C[0])].view(np.uint8),
                ],
                axis=1,
            )
        )
        in_maps.append({"w1q": w1q, "w2": w2c, "xT": xc, "boot": boot})
    return in_maps


def scatter_output(results, sizes, expert_of, C):
    """Gather per-core transposed outputs back into the full [T, H] f32 output."""
    tok_offs = np.concatenate([[0], np.cumsum(sizes)]).astype(np.int64)
    out = np.empty((T, H), np.float32)
    for c in range(NCORES):
        oT = np.asarray(results[c]["outT"])  # [P, sum(OO*Cj)] bf16
        off = 0
        for s, e in enumerate(expert_of[c]):
            n = int(sizes[e])
            Cj = int(C[s])
            # device writes one [OO, NB] block per nb-chunk
            chunks = []
            for nb in range(0, Cj, NMAX):
                NB = min(NMAX, Cj - nb)
                chunks.append(oT[:, off : off + OO * NB].reshape(P, OO, NB))
                off += OO * NB
            blk = chunks[0] if len(chunks) == 1 else np.concatenate(chunks, axis=2)
            # out[t, oo*P+oi] = blk[oi, oo, t]
            out[tok_offs[e] : tok_offs[e] + n] = (
                blk.transpose(2, 1, 0).reshape(Cj, H)[:n].astype(np.float32)
            )
    return out


def kernel(x, w1, w2, tokens_per_expert):
    from concourse import bass2jax

    sizes = np.asarray(tokens_per_expert, np.int64)
    expert_of, C, offs = plan(sizes)
    s1 = _w1_scale(np.asarray(w1, np.float32)) if W1_FP8 else 1.0
    nc = _get_program(C, 1.0 / s1)
    in_maps = prepare_inputs(x, w1, w2, sizes, expert_of, C, offs, s1)
    results = bass2jax.run_bass_via_pjrt(nc, in_maps, n_cores=NCORES)
    return scatter_output(results, sizes, expert_of, C)
